# revision 6
# baseline (speedup 1.0000x reference)
"""DeepClusteringLoss Trainium2 kernel.

loss = (||V^T V||_F^2 - 2 ||V^T E||_F^2 + ||E^T E||_F^2) / (B*N)
summed over batch, with E = embeddings.reshape(B, N, D), V =
assignments.reshape(B, N, S), N = F*T.

Sharding: data-parallel over batch; each of the 8 cores handles one batch
element.  On-core, the combined matrix W = [V | E] (N x 44) is streamed
through the PE array in 1024 chunks of 128 rows, accumulating the full
Gram G = W^T W (44 x 44) in PSUM.  Even/odd chunks go to different PE
column-group halves (tile_position) so their matmuls overlap in the
array.  The per-core scalar partial loss = ||G||^2 - 4 ||B||^2
(B = V^T E block) is reduced on-device; the host sums the 8 partials
(the "all-reduce") and divides by B*N.
"""

import os
from contextlib import ExitStack

import numpy as np

import concourse.bacc as bacc
import concourse.mybir as mybir
import concourse.tile as tile
from concourse.bass_utils import run_bass_kernel_spmd

B, F, T, D, S = 8, 256, 512, 40, 4
N = F * T              # rows per core (131072)
SD = S + D             # 44 combined features
P = 128                # partitions / chunk rows
N_CHUNKS = N // P      # 1024 matmul chunks per core
N_CORES = 8

# chunks per block: small blocks at the start (fast pipeline fill) and at
# the end (short drain after the last DMA lands)
BLOCK_SCHEDULE = [16, 16, 32] + [64] * 14 + [32, 16, 16]
assert sum(BLOCK_SCHEDULE) == N_CHUNKS

# matmul dtype knob: float32 (exact, PE 4 cyc/row) or float16 (PE 1 cyc/row)
MM_DT_NAME = os.environ.get("KERNEL_MM_DT", "float16")
COL_TILE = os.environ.get("KERNEL_COL_TILE", "1") == "1"

_nc_cache = {}


def _build_nc(key):
    mm_dt_name, col_tile = key
    mm_dt = getattr(mybir.dt, mm_dt_name)
    f32 = mybir.dt.float32
    cast = mm_dt != f32

    nc = bacc.Bacc("TRN2", target_bir_lowering=False, debug=False)
    E = nc.dram_tensor("embeddings", (N, D), f32, kind="ExternalInput")
    V = nc.dram_tensor("assignments", (N, S), f32, kind="ExternalInput")
    OUT = nc.dram_tensor("partial", (1, 1), f32, kind="ExternalOutput")

    with tile.TileContext(nc) as tc, ExitStack() as ctx:
        io_pool = ctx.enter_context(tc.tile_pool(name="io", bufs=4))
        w_pool = ctx.enter_context(tc.tile_pool(name="w", bufs=4))
        psum_pool = ctx.enter_context(tc.tile_pool(name="ps", bufs=1, space="PSUM"))
        # even chunks accumulate into partitions [0:SD] (PE col groups 0-1),
        # odd chunks into [64:64+SD] (col groups 2-3)
        g_ps = psum_pool.tile([64 + SD if col_tile else SD, SD], f32, tag="g")

        chunk = 0          # global chunk counter
        r0 = 0
        nblocks = len(BLOCK_SCHEDULE)
        for blk, ub in enumerate(BLOCK_SCHEDULE):
            rows = P * ub
            e_ap = E[r0:r0 + rows, :].rearrange("(p u) d -> p (u d)", p=P)
            v_ap = V[r0:r0 + rows, :].rearrange("(p u) s -> p (u s)", p=P)
            r0 += rows
            e_t = io_pool.tile([P, ub * D], mm_dt, tag="e")
            v_t = io_pool.tile([P, ub * S], mm_dt, tag="v")
            if cast:
                nc.gpsimd.dma_start(out=e_t[:], in_=e_ap)
                nc.gpsimd.dma_start(out=v_t[:], in_=v_ap)
            else:
                nc.sync.dma_start(out=e_t[:], in_=e_ap)
                nc.sync.dma_start(out=v_t[:], in_=v_ap)

            # Interleave into per-chunk [V_u | E_u] blocks of 44 columns.
            w_t = w_pool.tile([P, ub * SD], mm_dt, tag="w")
            w3 = w_t[:].rearrange("p (u c) -> p u c", c=SD)
            nc.vector.tensor_copy(
                w3[:, :, S:SD], e_t[:].rearrange("p (u d) -> p u d", d=D)
            )
            nc.scalar.copy(
                w3[:, :, 0:S], v_t[:].rearrange("p (u s) -> p u s", s=S)
            )

            last_blk = blk == nblocks - 1
            for u in range(ub):
                wu = w_t[:, u * SD:(u + 1) * SD]
                if col_tile:
                    half = chunk % 2
                    out_ap = g_ps[64 * half:64 * half + SD, :]
                    nc.tensor.matmul(
                        out_ap, wu, wu,
                        start=(chunk < 2),
                        # even group is closed later by the fold-in matmul
                        stop=(last_blk and u == ub - 1),
                        tile_position=(0, 64 * half),
                        skip_group_check=True,
                    )
                else:
                    nc.tensor.matmul(
                        g_ps[:], wu, wu,
                        start=(chunk == 0),
                        stop=(last_blk and u == ub - 1),
                    )
                chunk += 1

        # Epilogue: partial = sum(G^2) - 4 * sum(B^2), B = G[0:S, S:SD]
        ep = ctx.enter_context(tc.tile_pool(name="ep", bufs=1))
        g_sb = ep.tile([SD, SD], f32, tag="gsb")
        if col_tile:
            # Fold the odd-half accumulator (partitions 64:108) into the
            # even half on the PE: G_even += I^T @ G_odd.  DVE lanes can't
            # shift partitions, but a matmul contraction can.
            from concourse.masks import make_identity
            ident = ep.tile([64 + SD, SD], f32, tag="id")
            make_identity(nc, ident[64:64 + SD, :])
            o_sb = ep.tile([64 + SD, SD], f32, tag="osb")
            nc.vector.tensor_copy(o_sb[64:64 + SD, :], g_ps[64:64 + SD, :])
            nc.tensor.matmul(
                g_ps[0:SD, :], ident[64:64 + SD, :], o_sb[64:64 + SD, :],
                start=False, stop=True,
                tile_position=(64, 0), skip_group_check=True,
            )
            nc.vector.tensor_copy(g_sb[:], g_ps[0:SD, :])
        else:
            nc.vector.tensor_copy(g_sb[:], g_ps[0:SD, :])
        g2 = ep.tile([SD, SD], f32, tag="g2")
        nc.vector.tensor_mul(g2[:], g_sb[:], g_sb[:])
        colsum = ep.tile([SD, 1], f32, tag="cs")
        nc.vector.reduce_sum(colsum[:], g2[:], axis=mybir.AxisListType.X)
        bcol = ep.tile([S, 1], f32, tag="bc")
        nc.vector.reduce_sum(bcol[:], g2[0:S, S:SD], axis=mybir.AxisListType.X)
        bneg = ep.tile([S, 1], f32, tag="bn")
        nc.vector.tensor_scalar_mul(bneg[:], bcol[:], -4.0)
        ones = ep.tile([SD, 1], f32, tag="on")
        nc.vector.memset(ones[:], 1.0)
        s_ps = psum_pool.tile([1, 1], f32, tag="s")
        nc.tensor.matmul(s_ps[:], colsum[:], ones[:], start=True, stop=False)
        nc.tensor.matmul(s_ps[:], bneg[:], ones[0:S, :], start=False, stop=True)
        res = ep.tile([1, 1], f32, tag="r")
        nc.vector.tensor_copy(res[:], s_ps[:])
        nc.sync.dma_start(out=OUT[:, :], in_=res[:])

    nc.finalize()
    return nc


def _get_nc():
    key = (MM_DT_NAME, COL_TILE)
    if key not in _nc_cache:
        _nc_cache[key] = _build_nc(key)
    return _nc_cache[key]


def _run(embeddings: np.ndarray, assignments: np.ndarray, trace: bool = False):
    nc = _get_nc()
    in_maps = []
    for i in range(N_CORES):
        in_maps.append({
            "embeddings": np.ascontiguousarray(
                embeddings[i].reshape(N, D).astype(np.float32, copy=False)),
            "assignments": np.ascontiguousarray(
                assignments[i].reshape(N, S).astype(np.float32, copy=False)),
        })
    res = run_bass_kernel_spmd(
        nc, in_maps, core_ids=list(range(N_CORES)), trace=trace
    )
    partials = [float(r["partial"][0, 0]) for r in res.results]
    total = np.float32(np.sum(np.asarray(partials, dtype=np.float64)) / (B * N))
    return np.asarray(total, dtype=np.float32), res


def kernel(embeddings: np.ndarray, assignments: np.ndarray) -> np.ndarray:
    out, _ = _run(embeddings, assignments, trace=False)
    return out
